# revision 1
# baseline (speedup 1.0000x reference)
"""TRN2 Bass kernel for the 3-way factorization-machine MLP (nn_CP_B_53669911331094).

Data-parallel over 8 NeuronCores: each core handles 1024 of the 8192 batch rows.
Pipeline per core (all matmuls fp16 with fp32 PSUM accumulation):
  gather (fp16 tables, indirect DMA) -> PE transpose -> projections (+bias)
  -> DRAM-bounce row replication -> DVE tensor-tensor feature products
  -> 96-block W1 matmul accumulation -> ReLU -> W2 -> tanh -> W3 -> +b3.
"""
import os
import numpy as np

LATENT = 128
RED = 64
B = 8192
NCORES = 8
BC = B // NCORES          # 1024 batch rows per core
NT = BC // 128            # 8 tiles of 128 rows
NH = BC // 512            # 2 free-dim halves for N=512 matmuls
KB = 3 * RED * RED // 128  # 96 feature k-blocks
TPB = RED * RED // 128     # 32 k-blocks per pair

_CACHE = {}


def _build_nc(phases=3, reps=1, var="full"):
    import concourse.bass as bass
    import concourse.bacc as bacc
    import concourse.mybir as mybir
    from concourse.tile import TileContext

    f16, f32, i32 = mybir.dt.float16, mybir.dt.float32, mybir.dt.int32
    Relu = mybir.ActivationFunctionType.Relu
    Tanh = mybir.ActivationFunctionType.Tanh

    nc = bacc.Bacc("TRN2", target_bir_lowering=False, debug=False,
                   num_devices=NCORES, num_swdge_queues=4)

    tab_u = nc.dram_tensor("tab_u", [100000, LATENT], f16, kind="ExternalInput")
    tab_i = nc.dram_tensor("tab_i", [100000, LATENT], f16, kind="ExternalInput")
    tab_t = nc.dram_tensor("tab_t", [1000, LATENT], f16, kind="ExternalInput")
    idx_d = nc.dram_tensor("idx", [3, 128, NT], i32, kind="ExternalInput")
    wproj_d = nc.dram_tensor("wproj", [128, 320], f16, kind="ExternalInput")
    bproj_d = nc.dram_tensor("bproj", [128, 3], f32, kind="ExternalInput")
    w1t_d = nc.dram_tensor("w1t", [128, KB * 256], f16, kind="ExternalInput")
    b1t_d = nc.dram_tensor("b1t", [128, 2], f32, kind="ExternalInput")
    w2t_d = nc.dram_tensor("w2t", [128, 256], f16, kind="ExternalInput")
    b2t_d = nc.dram_tensor("b2t", [128, 1], f32, kind="ExternalInput")
    w3t_d = nc.dram_tensor("w3t", [128, 1], f16, kind="ExternalInput")
    b3_d = nc.dram_tensor("b3", [1, 1], f32, kind="ExternalInput")
    eye_d = nc.dram_tensor("eye", [128, 128], f16, kind="ExternalInput")
    out_d = nc.dram_tensor("out", [1, BC], f32, kind="ExternalOutput")
    spills = [(nc.dram_tensor(f"spill_i{r}", [RED, BC], f16),
               nc.dram_tensor(f"spill_j{r}", [RED, BC], f16))
              for r in range(reps)]

    tabs = [tab_u, tab_i, tab_t]

    with TileContext(nc) as tc:
        with tc.tile_pool(name="const", bufs=1) as cp, \
             tc.tile_pool(name="work", bufs=1) as wp:
            # ---- resident constants ----
            w1t = cp.tile([128, KB * 256], f16, tag="w1t")
            nc.sync.dma_start(w1t[:], w1t_d[:])
            wproj = cp.tile([128, 320], f16, tag="wproj")
            nc.sync.dma_start(wproj[:], wproj_d[:])
            w2t = cp.tile([128, 256], f16, tag="w2t")
            nc.sync.dma_start(w2t[:], w2t_d[:])
            w3t = cp.tile([128, 1], f16, tag="w3t")
            nc.sync.dma_start(w3t[:], w3t_d[:])
            eye = cp.tile([128, 128], f16, tag="eye")
            nc.sync.dma_start(eye[:], eye_d[:])
            idx = cp.tile([128, 3 * NT], i32, tag="idx")
            idx_src = bass.AP(idx_d[:].tensor, 0,
                              [[NT, 128], [128 * NT, 3], [1, NT]])
            nc.sync.dma_start(idx[:], idx_src)

            # biases: engine-local copies (1-wait discipline)
            braw = cp.tile([128, 3], f32, tag="braw")
            nc.sync.dma_start(braw[:], bproj_d[:])
            bproj = cp.tile([128, 3], f32, tag="bproj")
            nc.vector.tensor_copy(bproj[:], braw[:])
            b1raw = cp.tile([128, 2], f32, tag="b1raw")
            nc.sync.dma_start(b1raw[:], b1t_d[:])
            b1t = cp.tile([128, 2], f32, tag="b1t")
            nc.scalar.copy(b1t[:], b1raw[:])
            b2raw = cp.tile([128, 1], f32, tag="b2raw")
            nc.sync.dma_start(b2raw[:], b2t_d[:])
            b2t = cp.tile([128, 1], f32, tag="b2t")
            nc.scalar.copy(b2t[:], b2raw[:])
            b3raw = cp.tile([1, 1], f32, tag="b3raw")
            nc.sync.dma_start(b3raw[:], b3_d[:])
            b3 = cp.tile([1, 1], f32, tag="b3")
            nc.vector.tensor_copy(b3[:], b3raw[:])

            # ---- big SBUF work tiles ----
            eT = [wp.tile([128, BC], f16, tag=f"eT{x}", name=f"eT{x}")
                  for x in range(3)]
            iT = wp.tile([64, BC], f16, tag="iT")
            jj = wp.tile([128, BC], f16, tag="jj")
            kk = wp.tile([128, BC], f16, tag="kk")
            h1 = [wp.tile([128, BC], f16, tag=f"h1{o}", name=f"h1{o}")
                  for o in range(2)]
            h2 = wp.tile([128, BC], f16, tag="h2")
            out_sb = wp.tile([1, BC], f32, tag="out_sb")

            for _rep in range(reps):
              # ---- phase 1: gathers + transposes + projections ----
                with tc.tile_pool(name=f"ps1_{_rep}", bufs=1, space="PSUM") as ps1, \
                   tc.tile_pool(name=f"gp_{_rep}", bufs=8) as gp:
                  pj = []  # projection psums
                  for x in range(3):
                      for t in range(NT):
                          g = gp.tile([128, 128], f16, tag="g")
                          if var == "nogather":
                              nc.sync.dma_start(g[:], tabs[x][0:128, :])
                          else:
                              nc.gpsimd.indirect_dma_start(
                                  out=g[:], out_offset=None, in_=tabs[x][:],
                                  in_offset=bass.IndirectOffsetOnAxis(
                                      ap=idx[:, x * NT + t: x * NT + t + 1], axis=0))
                          tp = ps1.tile([128, 128], f16, tag="tp", bufs=2)
                          nc.tensor.transpose(tp[:], g[:], eye[:])
                          nc.vector.tensor_copy(eT[x][:, t * 128:(t + 1) * 128], tp[:])
                      # projection for table x
                      if x == 0:
                          p = ps1.tile([64, BC], f32, tag=f"proj{x}", name=f"p{x}")
                          lhsT = wproj[:, 0:64]
                      else:
                          p = ps1.tile([128, BC], f32, tag=f"proj{x}", name=f"p{x}")
                          lhsT = wproj[:, 64 + (x - 1) * 128: 64 + x * 128]
                      for nh in range(NH):
                          nc.tensor.matmul(p[:, nh * 512:(nh + 1) * 512], lhsT,
                                           eT[x][:, nh * 512:(nh + 1) * 512],
                                           start=True, stop=True)
                      pj.append(p)
                  # bias add + cast to fp16
                  nc.vector.tensor_scalar_add(iT[:], pj[0][:], bproj[0:64, 0:1])
                  nc.vector.tensor_scalar_add(jj[:], pj[1][:], bproj[:, 1:2])
                  nc.vector.tensor_scalar_add(kk[:], pj[2][:], bproj[:, 2:3])

                if phases >= 2:
                  # spill plain rows for DRAM-bounce replication
                  spill_i, spill_j = spills[_rep]
                  nc.sync.dma_start(spill_i[:], iT[:])
                  nc.sync.dma_start(spill_j[:], jj[0:64, :])

                  # ---- phase 2: feature blocks + W1 accumulation ----
                  with tc.tile_pool(name=f"ps2_{_rep}", bufs=1, space="PSUM") as ps2, \
                       tc.tile_pool(name=f"rp_{_rep}", bufs=8) as rp, \
                       tc.tile_pool(name=f"fp_{_rep}", bufs=8) as fp, \
                       tc.tile_pool(name=f"pr_{_rep}", bufs=2) as pr:
                      w1ps = [[ps2.tile([128, 512], f32, tag=f"w1ps{o}{h}",
                                        name=f"w1ps{o}{h}")
                               for h in range(NH)] for o in range(2)]

                      def w1_mms(kb, ft, start, stop):
                          if var == "nomm" and not (start or stop):
                              return
                          for o in range(2):
                              for h in range(NH):
                                  nc.tensor.matmul(
                                      w1ps[o][h][:],
                                      w1t[:, kb * 256 + o * 128:
                                          kb * 256 + (o + 1) * 128],
                                      ft[:, h * 512:(h + 1) * 512],
                                      start=start, stop=stop)

                      for t in range(TPB):
                          rep = rp.tile([128, BC], f16, tag="rep")
                          if var == "norep":
                              nc.sync.dma_start(rep[0:64, :], spill_i[:])
                              nc.sync.dma_start(rep[64:128, :], spill_i[:])
                          else:
                              src = bass.AP(spill_i[:].tensor, 2 * t * BC,
                                            [[BC, 2], [0, 64], [1, BC]])
                              eng = nc.sync if t % 2 == 0 else nc.scalar
                              eng.dma_start(rep[:], src)
                          probe = pr.tile([1, 1], f16, tag="probe")
                          nc.vector.tensor_copy(probe[:], rep[0:1, 0:1])
                          ft = fp.tile([128, BC], f16, tag="ft")
                          if var != "nott":
                              nc.vector.tensor_mul(ft[:], rep[:], jj[:])    # ij
                              nc.vector.tensor_mul(rep[:], rep[:], kk[:])   # ik in-place
                          w1_mms(t, ft, t == 0, False)
                          w1_mms(TPB + t, rep, False, False)
                      for t in range(TPB):
                          rep = rp.tile([128, BC], f16, tag="rep")
                          if var == "norep":
                              nc.sync.dma_start(rep[0:64, :], spill_j[:])
                              nc.sync.dma_start(rep[64:128, :], spill_j[:])
                          else:
                              src = bass.AP(spill_j[:].tensor, 2 * t * BC,
                                            [[BC, 2], [0, 64], [1, BC]])
                              eng = nc.sync if t % 2 == 0 else nc.scalar
                              eng.dma_start(rep[:], src)
                          probe = pr.tile([1, 1], f16, tag="probe")
                          nc.vector.tensor_copy(probe[:], rep[0:1, 0:1])
                          if var != "nott":
                              nc.vector.tensor_mul(rep[:], rep[:], kk[:])   # jk in-place
                          w1_mms(2 * TPB + t, rep, False, t == TPB - 1)

                      # ---- phase 3: MLP head ----
                      for o in range(2):
                          for h in range(NH):
                              nc.scalar.activation(
                                  h1[o][:, h * 512:(h + 1) * 512],
                                  w1ps[o][h][:], Relu,
                                  bias=b1t[:, o:o + 1], scale=1.0)
                      for h in range(NH):
                          p2 = ps2.tile([128, 512], f32, tag="w2ps", name="p2")
                          nc.tensor.matmul(p2[:], w2t[:, 0:128],
                                           h1[0][:, h * 512:(h + 1) * 512],
                                           start=True, stop=False)
                          nc.tensor.matmul(p2[:], w2t[:, 128:256],
                                           h1[1][:, h * 512:(h + 1) * 512],
                                           start=False, stop=True)
                          nc.scalar.activation(h2[:, h * 512:(h + 1) * 512], p2[:],
                                               Tanh, bias=b2t[:, 0:1], scale=1.0)
                      for h in range(NH):
                          p3 = ps2.tile([1, 512], f32, tag="w3ps", name="p3")
                          nc.tensor.matmul(p3[:], w3t[:],
                                           h2[:, h * 512:(h + 1) * 512],
                                           start=True, stop=True)
                          nc.vector.tensor_scalar_add(
                              out_sb[:, h * 512:(h + 1) * 512], p3[:], b3[0:1, 0:1])
                else:
                  # phase-1 bisect mode: dump a row of jj as the output
                  nc.vector.tensor_copy(out_sb[0:1, :], jj[0:1, :])

            nc.sync.dma_start(out_d[:], out_sb[:])
    nc.compile()
    return nc


def _prep_consts(user_emb, item_emb, time_emb, Wi, bi, Wj, bj, Wk, bk,
                 W1, b1, W2, b2, W3, b3):
    f16 = np.float16
    c = {}
    c["tab_u"] = np.ascontiguousarray(user_emb, dtype=f16)
    c["tab_i"] = np.ascontiguousarray(item_emb, dtype=f16)
    c["tab_t"] = np.ascontiguousarray(time_emb, dtype=f16)
    wproj = np.zeros((128, 320), f16)
    wproj[:, 0:64] = Wi.T
    wproj[:, 64:128] = Wj.T
    wproj[:, 128:192] = Wj.T
    wproj[:, 192:256] = Wk.T
    wproj[:, 256:320] = Wk.T
    c["wproj"] = wproj
    bproj = np.zeros((128, 3), np.float32)
    bproj[0:64, 0] = bi
    bproj[:, 1] = np.concatenate([bj, bj])
    bproj[:, 2] = np.concatenate([bk, bk])
    c["bproj"] = bproj
    # W1 [256, 12288] -> lhsT layout [128, 96*256]: block kb = W1.T[kb*128:(kb+1)*128, :]
    c["w1t"] = np.ascontiguousarray(
        W1.T.reshape(KB, 128, 256).transpose(1, 0, 2).reshape(128, KB * 256),
        dtype=f16)
    c["b1t"] = np.ascontiguousarray(b1.reshape(2, 128).T, dtype=np.float32)
    c["w2t"] = np.ascontiguousarray(
        W2.T.reshape(2, 128, 128).transpose(1, 0, 2).reshape(128, 256), dtype=f16)
    c["b2t"] = np.ascontiguousarray(b2.reshape(128, 1), dtype=np.float32)
    c["w3t"] = np.ascontiguousarray(W3.T, dtype=f16)
    c["b3"] = np.ascontiguousarray(b3.reshape(1, 1), dtype=np.float32)
    c["eye"] = np.eye(128, dtype=f16)
    return c


def _make_in_maps(consts, i_input, j_input, k_input):
    ii = np.asarray(i_input).astype(np.int32)
    jjx = np.asarray(j_input).astype(np.int32)
    kkx = np.asarray(k_input).astype(np.int32)
    in_maps = []
    for c in range(NCORES):
        sl = slice(c * BC, (c + 1) * BC)
        idx = np.stack([
            ii[sl].reshape(NT, 128).T,
            jjx[sl].reshape(NT, 128).T,
            kkx[sl].reshape(NT, 128).T,
        ]).astype(np.int32)  # [3, 128, NT]
        m = dict(consts)
        m["idx"] = np.ascontiguousarray(idx)
        in_maps.append(m)
    return in_maps


def kernel(i_input, j_input, k_input, user_emb, item_emb, time_emb,
           Wi, bi, Wj, bj, Wk, bk, W1, b1, W2, b2, W3, b3):
    from concourse.bass_utils import run_bass_kernel_spmd

    consts = _prep_consts(
        np.asarray(user_emb), np.asarray(item_emb), np.asarray(time_emb),
        np.asarray(Wi), np.asarray(bi), np.asarray(Wj), np.asarray(bj),
        np.asarray(Wk), np.asarray(bk), np.asarray(W1), np.asarray(b1),
        np.asarray(W2), np.asarray(b2), np.asarray(W3), np.asarray(b3))

    phases = int(os.environ.get("BASS_PHASES", "3"))
    reps = int(os.environ.get("BASS_REPS", "1"))
    key = ("nc", phases, reps)
    if key not in _CACHE:
        _CACHE[key] = _build_nc(phases, reps)
    nc = _CACHE[key]

    in_maps = _make_in_maps(consts, i_input, j_input, k_input)
    res = run_bass_kernel_spmd(nc, in_maps, list(range(NCORES)))
    out = np.concatenate([res.results[c]["out"][0] for c in range(NCORES)])
    return out.astype(np.float32)

